# revision 1
# baseline (speedup 1.0000x reference)
"""Causal single-head attention (B=4, T=2048, D=1024, fp32) on 8 TRN2 cores.

Sharding: core c -> (batch b = c//2, parity h = c%2). Each core computes the
output rows for query tiles qt in {2j + h : j=0..7} of its batch (zigzag
interleave of 128-row tiles, which balances the causal triangle between the
two cores sharing a batch). All 8 cores run ONE SPMD program; the parity
enters only through the data (host-gathered query stripes + mask content).

Per-core device work, all matmuls in fp32r (full-rate fp32 PE mode):
  Phase A: Q^T projection for the core's 8 query tiles (layout [e, q_local]).
  Phase B: loop over 4 key blocks of 512 keys: project K^T block and V block
           from a streamed slab of x^T, then for each live query tile:
           S = (Q^T)^T K^T in PSUM, add causal mask on diagonal blocks,
           P = exp(S/32) with row-sum accum, P^T via PE transposes,
           ctx += (P^T)^T V, accumulated in SBUF across key blocks.
  Phase C: ctx * (1/rowsum) via per-partition ACT scale, DMA out.

Host glue: transposes x/W once (DMA-efficient layouts), gathers the zigzag
query stripes, builds the parity-encoded causal mask, reassembles the output,
and adds bv at the end (softmax rows sum to 1, so ctx = P@(V+bv) = P@V + bv).
"""

import sys

sys.path.insert(0, "/opt/trn_rl_repo")

import numpy as np

import concourse.mybir as mybir
import concourse.tile as tile
from concourse import bacc
from concourse.bass_utils import run_bass_kernel_spmd
from concourse.masks import make_identity

N_CORES = 8
B, T, D = 4, 2048, 1024
P = 128
DC = D // P  # 8 contraction chunks
EC = D // P  # 8 output-feature chunks
KBW = 512  # key-block width
NKB = T // KBW  # 4 key blocks
NT = 8  # query tiles per core (of 16 per batch)
NQ = NT * P  # 1024 query rows per core
NEG = -1e30
SCALE = 1.0 / 32.0  # 1/sqrt(D)
MW = 6 * P  # mask width (512 + 2*128)

F32 = mybir.dt.float32
F32R = mybir.dt.float32r
ID = mybir.ActivationFunctionType.Identity
EXP = mybir.ActivationFunctionType.Exp


def build():
    nc = bacc.Bacc(
        "TRN2", target_bir_lowering=False, debug=False, num_devices=N_CORES
    )
    xT = nc.dram_tensor(
        "xT", [DC * NKB * P, KBW], F32R, kind="ExternalInput"
    ).ap()
    xqT = nc.dram_tensor(
        "xqT", [DC * (NQ // KBW) * P, KBW], F32R, kind="ExternalInput"
    ).ap()
    wqT = nc.dram_tensor("wqT", [D, D], F32R, kind="ExternalInput").ap()
    wkT = nc.dram_tensor("wkT", [D, D], F32R, kind="ExternalInput").ap()
    wvT = nc.dram_tensor("wvT", [D, D], F32R, kind="ExternalInput").ap()
    bq = nc.dram_tensor("bq", [D], F32, kind="ExternalInput").ap()
    bk = nc.dram_tensor("bk", [D], F32, kind="ExternalInput").ap()
    cmask = nc.dram_tensor("cmask", [P, MW], F32, kind="ExternalInput").ap()
    out = nc.dram_tensor("out", [NQ, D], F32, kind="ExternalOutput").ap()

    xT_v = xT.rearrange("(dc kb p) c -> dc kb p c", dc=DC, kb=NKB)
    xqT_v = xqT.rearrange("(dc s p) c -> dc s p c", dc=DC, s=NQ // KBW)
    wq_v = wqT.rearrange("(dc p) e -> p dc e", p=P)
    wk_v = wkT.rearrange("(dc p) e -> p dc e", p=P)
    wv_v = wvT.rearrange("(dc p) e -> p dc e", p=P)

    with tile.TileContext(nc) as tc:
        with (
            tc.tile_pool(name="const", bufs=1) as const,
            tc.tile_pool(name="w", bufs=1) as wpool,
            tc.tile_pool(name="slab", bufs=2) as slab,
            tc.tile_pool(name="big", bufs=1) as big,
            tc.tile_pool(name="p", bufs=2) as ppool,
            tc.tile_pool(name="pt", bufs=2) as ptpool,
            tc.tile_pool(name="fin", bufs=1) as fin,
            tc.tile_pool(name="psA", bufs=2, space="PSUM") as psA,
            tc.tile_pool(name="psS", bufs=2, space="PSUM") as psS,
            tc.tile_pool(name="psT", bufs=1, space="PSUM") as psT,
            tc.tile_pool(name="psC", bufs=1, space="PSUM") as psC,
        ):
            dma_rr = [0]

            def load_chunked(dst, view):
                # per-dc chunk DMAs, rotated across the DMA issue queues:
                # cuts first-use latency and spreads descriptor work
                engs = (nc.sync, nc.scalar, nc.gpsimd)
                for dc in range(DC):
                    eng = engs[dma_rr[0] % 3]
                    dma_rr[0] += 1
                    eng.dma_start(out=dst[:, dc, :], in_=view[:, dc, :])

            # ---- constants ----
            ident = const.tile([P, P], F32)
            make_identity(nc, ident)
            bigmask = const.tile([P, MW], F32)
            nc.gpsimd.dma_start(out=bigmask, in_=cmask)
            bq_sb = const.tile([P, EC], F32)
            nc.gpsimd.dma_start(out=bq_sb, in_=bq.rearrange("(c p) -> p c", p=P))
            bk_sb = const.tile([P, EC], F32)
            nc.gpsimd.dma_start(out=bk_sb, in_=bk.rearrange("(c p) -> p c", p=P))

            # persistent state
            qT_sb = big.tile([P, EC, NQ], F32R)  # Q^T, [e, local q]
            rs = big.tile([P, NT * NKB], F32)  # per (q-tile, kb) exp row-sums
            ctx_acc = [
                big.tile([P, D], F32, tag=f"ctx{j}", name=f"ctx{j}")
                for j in range(NT)
            ]

            wq_sb = wpool.tile([P, DC, D], F32R, tag="w0")

            # ---- Phase A: Q^T projection ----
            for s in range(NQ // KBW):
                sl = slab.tile([P, DC, KBW], F32R, tag="slab")
                if s == 0:
                    for dc in range(DC):
                        nc.sync.dma_start(out=sl[:, dc, :], in_=xqT_v[dc, 0])
                        nc.scalar.dma_start(
                            out=wq_sb[:, dc, :], in_=wq_v[:, dc, :]
                        )
                else:
                    for dc in range(DC):
                        eng = (nc.sync, nc.scalar, nc.gpsimd)[dma_rr[0] % 3]
                        dma_rr[0] += 1
                        eng.dma_start(out=sl[:, dc, :], in_=xqT_v[dc, s])
                for ec in range(EC):
                    ps = psA.tile([P, KBW], F32, tag="proj")
                    for dc in range(DC):
                        nc.tensor.matmul(
                            ps,
                            wq_sb[:, dc, ec * P : (ec + 1) * P],
                            sl[:, dc, :],
                            start=(dc == 0),
                            stop=(dc == DC - 1),
                        )
                    nc.scalar.activation(
                        out=qT_sb[:, ec, s * KBW : (s + 1) * KBW],
                        in_=ps,
                        func=ID,
                        bias=bq_sb[:, ec : ec + 1],
                    )

            wk_sb = wpool.tile([P, DC, D], F32R, tag="w1")
            wv_sb = None

            # ---- Phase B: key blocks ----
            sl = slab.tile([P, DC, KBW], F32R, tag="slab", name="sl0")
            # interleave the two inputs K-proj needs first
            for dc in range(DC):
                nc.sync.dma_start(out=sl[:, dc, :], in_=xT_v[dc, 0])
                nc.scalar.dma_start(out=wk_sb[:, dc, :], in_=wk_v[:, dc, :])
            for kb in range(NKB):
                kT = big.tile([P, EC, KBW], F32R, tag="kT")
                for ec in range(EC):
                    ps = psA.tile([P, KBW], F32, tag="proj")
                    for dc in range(DC):
                        nc.tensor.matmul(
                            ps,
                            wk_sb[:, dc, ec * P : (ec + 1) * P],
                            sl[:, dc, :],
                            start=(dc == 0),
                            stop=(dc == DC - 1),
                        )
                    nc.scalar.activation(
                        out=kT[:, ec, :], in_=ps, func=ID, bias=bk_sb[:, ec : ec + 1]
                    )

                if kb == 0:
                    wv_sb = wpool.tile([P, DC, D], F32R, tag="w0")  # wq slot
                    load_chunked(wv_sb, wv_v)
                v = big.tile([P, 4, D], F32R, tag="v")
                sl_next = (
                    slab.tile([P, DC, KBW], F32R, tag="slab", name=f"sl{kb + 1}")
                    if kb + 1 < NKB
                    else None
                )
                for tcc in range(4):
                    for ev in range(2):
                        ps = psA.tile([P, KBW], F32, tag="proj")
                        for dc in range(DC):
                            nc.tensor.matmul(
                                ps,
                                sl[:, dc, tcc * P : (tcc + 1) * P],
                                wv_sb[:, dc, ev * KBW : (ev + 1) * KBW],
                                start=(dc == 0),
                                stop=(dc == DC - 1),
                            )
                        nc.scalar.activation(
                            out=v[:, tcc, ev * KBW : (ev + 1) * KBW], in_=ps, func=ID
                        )
                if sl_next is not None:
                    for dc in range(DC):
                        eng = (nc.sync, nc.scalar, nc.gpsimd)[dma_rr[0] % 3]
                        dma_rr[0] += 1
                        eng.dma_start(out=sl_next[:, dc, :], in_=xT_v[dc, kb + 1])

                for j in range(NT):
                    if j // 2 < kb:  # this query tile ends before this block
                        continue
                    ntcc = 2 * (j % 2) + 2 if kb == j // 2 else 4
                    W = ntcc * P  # keys beyond W in this block are fully masked
                    ps_s = psS.tile([P, KBW], F32, tag="S")
                    for ec in range(EC):
                        nc.tensor.matmul(
                            ps_s[:, :W],
                            qT_sb[:, ec, j * P : (j + 1) * P],
                            kT[:, ec, :W],
                            start=(ec == 0),
                            stop=(ec == EC - 1),
                        )
                    if j // 2 == kb:  # diagonal block: causal mask
                        moff = (2 - 2 * (j % 2)) * P
                        nc.vector.tensor_add(
                            ps_s[:, :W], ps_s[:, :W], bigmask[:, moff : moff + W]
                        )
                    p_sb = ppool.tile([P, KBW], F32R, tag="p")
                    nc.scalar.activation(
                        out=p_sb[:, :W],
                        in_=ps_s[:, :W],
                        func=EXP,
                        scale=SCALE,
                        accum_out=rs[:, j * NKB + kb : j * NKB + kb + 1],
                    )
                    ps_t = psT.tile([P, KBW], F32, tag="pt")
                    for tcc in range(ntcc):
                        nc.tensor.matmul(
                            ps_t[:, tcc * P : (tcc + 1) * P],
                            p_sb[:, tcc * P : (tcc + 1) * P].bitcast(F32),
                            ident,
                            is_transpose=True,
                            start=True,
                            stop=True,
                        )
                    pT_sb = ptpool.tile([P, KBW], F32R, tag="pT")
                    nc.scalar.activation(
                        out=pT_sb[:, : ntcc * P], in_=ps_t[:, : ntcc * P], func=ID
                    )
                    ps_c = psC.tile([P, D], F32, tag="ctx", name="ps_c")
                    for ev in range(2):
                        sli = slice(ev * KBW, (ev + 1) * KBW)
                        for tcc in range(ntcc):
                            nc.tensor.matmul(
                                ps_c[:, sli],
                                pT_sb[:, tcc * P : (tcc + 1) * P],
                                v[:, tcc, sli],
                                start=(tcc == 0),
                                stop=(tcc == ntcc - 1),
                            )
                        if kb == 0:
                            nc.vector.tensor_copy(ctx_acc[j][:, sli], ps_c[:, sli])
                        else:
                            nc.vector.tensor_add(
                                ctx_acc[j][:, sli], ctx_acc[j][:, sli], ps_c[:, sli]
                            )

                    if kb == j // 2:  # last key block: normalize + store now
                        nkb = j // 2 + 1
                        rt = fin.tile([P, 1], F32, tag="rt", name="rt")
                        nc.vector.reduce_sum(
                            rt,
                            rs[:, j * NKB : j * NKB + nkb],
                            axis=mybir.AxisListType.X,
                        )
                        rc = fin.tile([P, 1], F32, tag="rc", name="rc")
                        nc.vector.reciprocal(rc, rt)
                        ob = fin.tile([P, D], F32, tag="ob", name="ob")
                        for ev in range(2):
                            sli = slice(ev * KBW, (ev + 1) * KBW)
                            nc.scalar.activation(
                                out=ob[:, sli], in_=ctx_acc[j][:, sli],
                                func=ID, scale=rc,
                            )
                            nc.sync.dma_start(
                                out=out[j * P : (j + 1) * P, sli], in_=ob[:, sli]
                            )

                sl = sl_next

    nc.compile()
    return nc


_cache = {}


def _get_nc():
    if "nc" not in _cache:
        _cache["nc"] = build()
    return _cache["nc"]


def _host_mask(h: int) -> np.ndarray:
    # mask[i, u] = 0 where u <= i + 256 + 128*h else NEG; sliced on-device at
    # offset (2 - 2*(j%2))*128 this yields the causal mask for qt = 2j + h.
    i = np.arange(P)[:, None]
    u = np.arange(MW)[None, :]
    return np.where(u <= i + 2 * P + h * P, 0.0, NEG).astype(np.float32)


def run(inputs, trace: bool = False):
    """Returns (output [B,T,D] fp32, BassKernelResults)."""
    nc = _get_nc()
    x = np.asarray(inputs["x"], dtype=np.float32)
    bq = np.asarray(inputs["bq"], dtype=np.float32)
    bk = np.asarray(inputs["bk"], dtype=np.float32)
    bv = np.asarray(inputs["bv"], dtype=np.float32)
    wqT = np.ascontiguousarray(np.asarray(inputs["Wq"], dtype=np.float32).T)
    wkT = np.ascontiguousarray(np.asarray(inputs["Wk"], dtype=np.float32).T)
    wvT = np.ascontiguousarray(np.asarray(inputs["Wv"], dtype=np.float32).T)
    xT = np.transpose(x, (0, 2, 1))  # [B, D, T]
    # chunk-major: [dc, kb, p, c] contiguous per (dc, kb) 256KB chunk
    xTc = np.ascontiguousarray(
        xT.reshape(B, DC, P, NKB, KBW).transpose(0, 1, 3, 2, 4)
    ).reshape(B, DC * NKB * P, KBW)

    masks = [_host_mask(0), _host_mask(1)]
    in_maps = []
    for c in range(N_CORES):
        b, h = c // 2, c % 2
        qcols = (
            np.arange(NQ) // P * 2 * P + h * P + np.arange(NQ) % P
        )  # global t of local q
        xq = xT[b][:, qcols]  # [D, NQ]
        xqc = np.ascontiguousarray(
            xq.reshape(DC, P, NQ // KBW, KBW).transpose(0, 2, 1, 3)
        ).reshape(DC * (NQ // KBW) * P, KBW)
        in_maps.append(
            {
                "xT": xTc[b],
                "xqT": xqc,
                "wqT": wqT,
                "wkT": wkT,
                "wvT": wvT,
                "bq": bq,
                "bk": bk,
                "cmask": masks[h],
            }
        )

    res = run_bass_kernel_spmd(
        nc, in_maps, core_ids=list(range(N_CORES)), trace=trace
    )

    out = np.empty((B, T, D), dtype=np.float32)
    for c in range(N_CORES):
        b, h = c // 2, c % 2
        o = res.results[c]["out"]  # [NQ, D]
        for j in range(NT):
            qt = 2 * j + h
            out[b, qt * P : (qt + 1) * P, :] = o[j * P : (j + 1) * P, :]
    out += bv  # softmax rows sum to 1, so bv folds out of the attention
    return out, res


def kernel(**inputs) -> np.ndarray:
    out, _ = run(inputs)
    return out



# revision 3
# speedup vs baseline: 1.0310x; 1.0310x over previous
"""Causal single-head attention (B=4, T=2048, D=1024) on 8 TRN2 cores, v2.

Sharding: core c -> (batch b = c//2, parity h = c%2). The PAIR splits the
KEYS zigzag: core h owns key chunks {128*(2i+h) : i=0..7} (1024 keys), so
K/V projection is computed once per pair (split), while Q projection for
all 2048 queries is duplicated (cheaper than duplicating K+V).

S^T formulation: scores are computed transposed, S^T[k, q] (keys on PSUM
partitions), which (a) removes all PE transposes of P, (b) lets V be used
in natural [k, d] layout, (c) lets ctx[q, d] accumulate across key chunks
entirely in PSUM (no vector adds). exp(S^T) goes straight to SBUF as P^T
(bf16). Row sums (over k = partitions) use a ones-vector matmul per chunk,
accumulated into SBUF segments by the DVE.

Each core outputs UNNORMALIZED partial ctx (bf16) + partial row sums
(f32); the host merges: out = (ctx_e + ctx_o) / (rs_e + rs_o) + bv.

All matmul operands are bf16 (same PE rate as fp32r at wide tiles, no
narrow-tile penalty, half the SBUF/DMA), accumulation stays fp32 in PSUM.

One SPMD program; parity enters only through data (host-gathered key
columns xTk + parity mask256 + host reassembly).
"""

import sys

sys.path.insert(0, "/opt/trn_rl_repo")

import ml_dtypes
import numpy as np

import concourse.mybir as mybir
import concourse.tile as tile
from concourse import bacc
from concourse.bass_utils import run_bass_kernel_spmd

N_CORES = 8
B, T, D = 4, 2048, 1024
P = 128
DC = D // P  # 8 contraction chunks
EC = D // P  # 8 output-feature chunks
NG = T // 512  # 4 query groups of 512
NCH = 8  # key chunks per core (of 16 per batch)
NEG = -1e30
SCALE = 1.0 / 32.0  # 1/sqrt(D)

F32 = mybir.dt.float32
BF16 = mybir.dt.bfloat16
ID = mybir.ActivationFunctionType.Identity
EXP = mybir.ActivationFunctionType.Exp


def slot_width(g: int, i: int) -> int:
    # window of slot (g, i) covers queries [512*(g+1) - W, 512*(g+1))
    return min(512, 512 * (g + 1) - 256 * i)


def build():
    nc = bacc.Bacc(
        "TRN2", target_bir_lowering=False, debug=False, num_devices=N_CORES
    )
    xTq = nc.dram_tensor(
        "xTq", [DC * NG * P, 512], BF16, kind="ExternalInput"
    ).ap()
    xTk = nc.dram_tensor("xTk", [DC * P, 1024], BF16, kind="ExternalInput").ap()
    wqT = nc.dram_tensor("wqT", [D, D], BF16, kind="ExternalInput").ap()
    wkT = nc.dram_tensor("wkT", [D, D], BF16, kind="ExternalInput").ap()
    wvT = nc.dram_tensor("wvT", [D, D], BF16, kind="ExternalInput").ap()
    bq = nc.dram_tensor("bq", [D], F32, kind="ExternalInput").ap()
    bk = nc.dram_tensor("bk", [D], F32, kind="ExternalInput").ap()
    mask256 = nc.dram_tensor("mask256", [P, 256], F32, kind="ExternalInput").ap()
    ones = nc.dram_tensor("ones", [P, 1], BF16, kind="ExternalInput").ap()
    ctx_out = nc.dram_tensor("ctx_out", [T, D], BF16, kind="ExternalOutput").ap()
    rs_out = nc.dram_tensor("rs_out", [1, T], F32, kind="ExternalOutput").ap()

    xTq_v = xTq.rearrange("(dc s p) c -> dc s p c", dc=DC, s=NG)
    xTq_s = xTq.rearrange("(dc s p) c -> s p dc c", dc=DC, s=NG)
    xTk_v = xTk.rearrange("(dc p) k -> p dc k", p=P)
    wq_v = wqT.rearrange("(dc p) e -> p dc e", p=P)
    wk_v = wkT.rearrange("(dc p) e -> p dc e", p=P)
    wv_v = wvT.rearrange("(dc p) e -> p dc e", p=P)

    with tile.TileContext(nc) as tc:
        with (
            tc.tile_pool(name="const", bufs=1) as const,
            tc.tile_pool(name="w", bufs=1) as wpool,
            tc.tile_pool(name="xk", bufs=1) as xkpool,
            tc.tile_pool(name="slab", bufs=2) as slab,
            tc.tile_pool(name="big", bufs=1) as big,
            tc.tile_pool(name="pt", bufs=14) as ptpool,
            tc.tile_pool(name="ob", bufs=2) as obpool,
            tc.tile_pool(name="psProj", bufs=2, space="PSUM") as psProj,
            tc.tile_pool(name="psS", bufs=2, space="PSUM") as psS,
            tc.tile_pool(name="psC", bufs=3, space="PSUM") as psC,
            tc.tile_pool(name="psRs", bufs=1, space="PSUM") as psRs,
        ):
            dma_rr = [0]
            engs = (nc.sync, nc.scalar, nc.gpsimd)

            def rr_dma(dst, src):
                eng = engs[dma_rr[0] % 3]
                dma_rr[0] += 1
                eng.dma_start(out=dst, in_=src)

            # ---- constants / weights / key-column slab ----
            ones_sb = const.tile([P, 1], BF16)
            nc.gpsimd.dma_start(out=ones_sb, in_=ones)
            m256 = const.tile([P, 256], F32)
            nc.gpsimd.dma_start(out=m256, in_=mask256)
            bq_sb = const.tile([P, EC], F32)
            nc.gpsimd.dma_start(out=bq_sb, in_=bq.rearrange("(c p) -> p c", p=P))
            bk_sb = const.tile([P, EC], F32)
            nc.gpsimd.dma_start(out=bk_sb, in_=bk.rearrange("(c p) -> p c", p=P))

            wk_sb = wpool.tile([P, DC, D], BF16, tag="wk")
            xk_sb = xkpool.tile([P, DC, 1024], BF16)
            for dc in range(DC):
                nc.sync.dma_start(out=xk_sb[:, dc, :], in_=xTk_v[:, dc, :])
                nc.scalar.dma_start(out=wk_sb[:, dc, :], in_=wk_v[:, dc, :])
            # late tensors (first used 30-65us in): one DMA instruction each
            # to cut semaphore/queue traffic
            wv_sb = wpool.tile([P, DC, D], BF16, tag="wv")
            wq_sb = wpool.tile([P, DC, D], BF16, tag="wq")
            sl0 = slab.tile([P, DC, 512], BF16, tag="slab", name="sl0")
            nc.gpsimd.dma_start(out=wv_sb, in_=wv_v[:, :, :])
            nc.scalar.dma_start(out=wq_sb, in_=wq_v[:, :, :])
            nc.sync.dma_start(out=sl0, in_=xTq_s[0])

            # persistent per-core state
            qT = big.tile([P, EC, T], BF16)  # Q^T for all queries
            kT = big.tile([P, EC, 1024], BF16)  # K^T for my 1024 keys
            v = big.tile([P, NCH, D], BF16)  # V for my keys, [k, chunk, d]
            rs_sb = big.tile([1, T], F32)  # row-sum segments

            # ---- K^T projection (my 1024 keys) ----
            for ec in range(EC):
                for kg in range(2):
                    ps = psProj.tile([P, 512], F32, tag="proj")
                    for dc in range(DC):
                        nc.tensor.matmul(
                            ps,
                            wk_sb[:, dc, ec * P : (ec + 1) * P],
                            xk_sb[:, dc, kg * 512 : (kg + 1) * 512],
                            start=(dc == 0),
                            stop=(dc == DC - 1),
                        )
                    nc.scalar.activation(
                        out=kT[:, ec, kg * 512 : (kg + 1) * 512],
                        in_=ps,
                        func=ID,
                        bias=bk_sb[:, ec : ec + 1],
                    )

            # ---- V projection (my keys; no bias, bv folds out on host) ----
            for i in range(NCH):
                for dh in range(2):
                    ps = psProj.tile([P, 512], F32, tag="proj")
                    for dc in range(DC):
                        nc.tensor.matmul(
                            ps,
                            xk_sb[:, dc, i * P : (i + 1) * P],
                            wv_sb[:, dc, dh * 512 : (dh + 1) * 512],
                            start=(dc == 0),
                            stop=(dc == DC - 1),
                        )
                    nc.scalar.activation(
                        out=v[:, i, dh * 512 : (dh + 1) * 512], in_=ps, func=ID
                    )

            sl = sl0
            for g in range(NG):
                # ---- Q^T projection for this group's 512 queries ----
                sl_next = (
                    slab.tile([P, DC, 512], BF16, tag="slab", name=f"sl{g + 1}")
                    if g + 1 < NG
                    else None
                )
                for ec in range(EC):
                    ps = psProj.tile([P, 512], F32, tag="proj")
                    for dc in range(DC):
                        nc.tensor.matmul(
                            ps,
                            wq_sb[:, dc, ec * P : (ec + 1) * P],
                            sl[:, dc, :],
                            start=(dc == 0),
                            stop=(dc == DC - 1),
                        )
                    nc.scalar.activation(
                        out=qT[:, ec, g * 512 : (g + 1) * 512],
                        in_=ps,
                        func=ID,
                        bias=bq_sb[:, ec : ec + 1],
                    )
                if sl_next is not None:
                    for dc in range(DC):
                        rr_dma(sl_next[:, dc, :], xTq_v[dc, g + 1])

                # ---- S^T + exp + row sums for this group's slots ----
                pts = []
                for i in range(2 * g + 2):
                    W = slot_width(g, i)
                    qlo = 512 * (g + 1) - W
                    ps_s = psS.tile([P, 512], F32, tag="S")
                    for ec in range(EC):
                        nc.tensor.matmul(
                            ps_s[:, :W],
                            kT[:, ec, i * P : (i + 1) * P],
                            qT[:, ec, qlo : qlo + W],
                            start=(ec == 0),
                            stop=(ec == EC - 1),
                        )
                    if i >= 2 * g:  # diagonal slot: causal (+parity pad) mask
                        nc.vector.tensor_add(
                            ps_s[:, :256], ps_s[:, :256], m256
                        )
                    pt = ptpool.tile([P, 512], BF16, tag="pT", name=f"pT_{g}_{i}")
                    nc.scalar.activation(
                        out=pt[:, :W], in_=ps_s[:, :W], func=EXP, scale=SCALE
                    )
                    pts.append(pt)

                # ---- ctx for this group's 4 query tiles ----
                for j in range(4 * g, 4 * g + 4):
                    ob = obpool.tile([P, D], BF16, tag="ob", name=f"ob{j}")
                    for dh in range(2):
                        ps_c = psC.tile([P, 512], F32, tag="ctx")
                        ni = j // 2 + 1  # slots 0..j//2 contribute
                        for i in range(ni):
                            W = slot_width(g, i)
                            offj = 128 * j - (512 * (g + 1) - W)
                            nc.tensor.matmul(
                                ps_c,
                                pts[i][:, offj : offj + P],
                                v[:, i, dh * 512 : (dh + 1) * 512],
                                start=(i == 0),
                                stop=(i == ni - 1),
                            )
                        nc.vector.tensor_copy(
                            ob[:, dh * 512 : (dh + 1) * 512], ps_c
                        )
                    nc.sync.dma_start(
                        out=ctx_out[j * P : (j + 1) * P, :], in_=ob
                    )

                # ---- row sums, AFTER ctx so the exp results are long done
                # and the PE queue never waits on the scalar engine. Slot
                # windows are suffixes of the group (slot 0 spans all 512),
                # so one PSUM-accumulating chain per group is valid.
                nslots = 2 * g + 2
                ps_r = psRs.tile([1, 512], F32, tag="rs")
                for i in range(nslots):
                    W = slot_width(g, i)
                    nc.tensor.matmul(
                        ps_r[:1, 512 - W : 512],
                        ones_sb,
                        pts[i][:, :W],
                        start=(i == 0),
                        stop=(i == nslots - 1),
                        skip_group_check=True,
                    )
                nc.vector.tensor_copy(
                    rs_sb[:, g * 512 : (g + 1) * 512], ps_r[:1, :512]
                )
                sl = sl_next

            nc.gpsimd.dma_start(out=rs_out, in_=rs_sb)

    nc.compile()
    return nc


_cache = {}


def _get_nc():
    if "nc" not in _cache:
        _cache["nc"] = build()
    return _cache["nc"]


def _host_mask(h: int) -> np.ndarray:
    # mask256[k, c]: applied at window cols [0, 256) of each diagonal slot.
    # h=0: [tri | zeros]; h=1: [all-NEG | tri] (pad block kills the extra
    # 128 query cols the parity padding introduces).
    kk = np.arange(P)[:, None]
    cc = np.arange(256)[None, :]
    qq = cc - 128 * h  # query index relative to the chunk's diagonal
    return np.where(kk <= qq, 0.0, NEG).astype(np.float32)


def run(inputs, trace: bool = False):
    """Returns (output [B,T,D] fp32, BassKernelResults)."""
    nc = _get_nc()
    x = np.asarray(inputs["x"], dtype=np.float32)
    bqh = np.asarray(inputs["bq"], dtype=np.float32)
    bkh = np.asarray(inputs["bk"], dtype=np.float32)
    bvh = np.asarray(inputs["bv"], dtype=np.float32)
    wqT = np.ascontiguousarray(
        np.asarray(inputs["Wq"], dtype=np.float32).T.astype(ml_dtypes.bfloat16)
    )
    wkT = np.ascontiguousarray(
        np.asarray(inputs["Wk"], dtype=np.float32).T.astype(ml_dtypes.bfloat16)
    )
    wvT = np.ascontiguousarray(
        np.asarray(inputs["Wv"], dtype=np.float32).T.astype(ml_dtypes.bfloat16)
    )
    xT = np.transpose(x, (0, 2, 1)).astype(ml_dtypes.bfloat16)  # [B, D, T]
    # natural slab layout [dc, s, p, c]
    xTq_h = np.ascontiguousarray(
        xT.reshape(B, DC, P, NG, 512).transpose(0, 1, 3, 2, 4)
    ).reshape(B, DC * NG * P, 512)

    cols = [
        (np.arange(1024) // P * 2 * P + h * P + np.arange(1024) % P)
        for h in range(2)
    ]
    masks = [_host_mask(0), _host_mask(1)]
    ones_h = np.ones((P, 1), dtype=ml_dtypes.bfloat16)

    in_maps = []
    for c in range(N_CORES):
        b, h = c // 2, c % 2
        xk = np.ascontiguousarray(xT[b][:, cols[h]]).reshape(DC * P, 1024)
        in_maps.append(
            {
                "xTq": xTq_h[b],
                "xTk": xk,
                "wqT": wqT,
                "wkT": wkT,
                "wvT": wvT,
                "bq": bqh,
                "bk": bkh,
                "mask256": masks[h],
                "ones": ones_h,
            }
        )

    res = run_bass_kernel_spmd(
        nc, in_maps, core_ids=list(range(N_CORES)), trace=trace
    )

    out = np.empty((B, T, D), dtype=np.float32)
    for b in range(B):
        ce = res.results[2 * b]["ctx_out"].astype(np.float32)
        co = res.results[2 * b + 1]["ctx_out"].astype(np.float32)
        rs = (
            res.results[2 * b]["rs_out"] + res.results[2 * b + 1]["rs_out"]
        ).reshape(T, 1)
        out[b] = (ce + co) / rs
    out += bvh
    return out, res


def kernel(**inputs) -> np.ndarray:
    out, _ = run(inputs)
    return out
